# revision 20
# baseline (speedup 1.0000x reference)
"""Trainium2 Bass kernel for causal GQA attention (B=1, T=4096, D=2048,
H=16, Hkv=4, Dh=128, RoPE) sharded over 8 NeuronCores.

Sharding: tensor-parallel over heads — each core owns 2 q-heads and the
kv head they share (core c: q-heads {2c, 2c+1}, kv head c//2). Each core
computes its q/k/v projections, RoPE, causal attention and its partial
o_proj contribution y_c = O_c @ Wo_c; the host sums the 8 partials.

v2 dataflow changes vs v1:
  - softmax denominators (l) no longer run as per-block full-ones
    matmuls (which cost as much PE time as the O accumulation itself).
    P tiles are retained in a 17-slot SBUF ring (pbuf) and l is computed
    in deferred half-window bursts of M=1 matmuls using PE column tiling
    (tile_position=(0,32j)): head x q-half -> 4 independent 32-col array
    tiles run concurrently, so l streams at 1/4 the column cost.
  - 1/l: lb PSUM rows -> SBUF copy (DVE) -> reciprocal (DVE) ->
    gpsimd partition_broadcast per (head, q-half) quarter -> f32 lbc,
    then O^T row normalize on DVE as before.
  - exp for both heads fused into one ACT instruction per kv block
    (S matmuls write one [128,2,512] PSUM tile = 2 banks), halving ACT's
    per-instruction overhead; ACT otherwise becomes the rate limiter
    once l's PE time is gone.
  - V^T -> V natural transposes moved from PE+PSUM to XBAR DMA
    transposes on the SP queue.
  - o_proj is a global work queue: one (row-tile, col-tile) matmul pair
    + drain is woven between attention blocks / proj m-groups, so PE
    has exp-independent work whenever ACT lags; backlog drains into the
    late (ACT-heavy) windows and a pipelined tail burst.
  - startup DMAs spread across gpsimd/sync/scalar queues.
"""

import sys

sys.path.insert(0, "/opt/trn_rl_repo")

import math
from collections import deque
from contextlib import ExitStack

import ml_dtypes
import numpy as np

import concourse.bass as bass
import concourse.tile as tile
from concourse import bacc, mybir
from concourse.bass_utils import run_bass_kernel_spmd
from concourse.masks import make_identity

BF16 = mybir.dt.bfloat16
F32 = mybir.dt.float32
NPBF16 = ml_dtypes.bfloat16

B, T, D = 1, 4096, 2048
H, HKV, DH = 16, 4, 128
GROUP = H // HKV
ROPE_BASE = 10000.0
N_CORES = 8
HL = H // N_CORES  # q-heads per core
KC = D // 128      # contraction tiles for projections
NQ = T // 512      # 512-wide q tiles
NJ = T // 128      # 128-wide kv tiles
NM = D // 512      # 512-wide output column tiles
NSLOT = 17         # P-tile ring slots (16-block half-window + 1 lookahead)
SCALE = 1.0 / math.sqrt(DH)

Exp = mybir.ActivationFunctionType.Exp
Copy = mybir.ActivationFunctionType.Copy


def _build(nc):
    xp = nc.dram_tensor("xp", [NQ, 128, KC, 512], BF16, kind="ExternalInput").ap()
    wqkv = nc.dram_tensor("wqkv", [128, KC, 4, 128], BF16, kind="ExternalInput").ap()
    wo = nc.dram_tensor("wo", [128, HL, D], BF16, kind="ExternalInput").ap()
    cos2 = nc.dram_tensor("cos2", [128, T], BF16, kind="ExternalInput").ap()
    sinsig = nc.dram_tensor("sinsig", [128, T], BF16, kind="ExternalInput").ap()
    y = nc.dram_tensor("y", [T, D], BF16, kind="ExternalOutput").ap()

    with tile.TileContext(nc) as tc, ExitStack() as ctx:
        const = ctx.enter_context(tc.tile_pool(name="const", bufs=1))
        xpool = ctx.enter_context(tc.tile_pool(name="xp", bufs=2))
        # PSUM: "s" 2x[128,2,512]f32 = 4 banks; "oacc" 2; "lacc" 1; "yp" 1.
        pspool = ctx.enter_context(tc.tile_pool(name="ps", bufs=2, space="PSUM"))
        opsum = ctx.enter_context(tc.tile_pool(name="ops", bufs=2, space="PSUM"))
        lpsum = ctx.enter_context(tc.tile_pool(name="lps", bufs=1, space="PSUM"))
        ypsum = ctx.enter_context(tc.tile_pool(name="yps", bufs=1, space="PSUM"))
        swpool = ctx.enter_context(tc.tile_pool(name="sw", bufs=2))
        lrowp = ctx.enter_context(tc.tile_pool(name="lr", bufs=3))
        lbcp = ctx.enter_context(tc.tile_pool(name="lbc", bufs=3))
        yrow = ctx.enter_context(tc.tile_pool(name="yr", bufs=4))

        wqkv_sb = const.tile([128, KC, 4, 128], BF16, tag="wqkv")
        wo_sb = const.tile([128, HL, D], BF16, tag="wo")
        cos_sb = const.tile([128, T], BF16, tag="cos")
        sin_sb = const.tile([128, T], BF16, tag="sin")
        ident = const.tile([128, 128], BF16, tag="ident")
        qkvT = const.tile([128, 4, T], BF16, tag="qkvT")   # Q0,Q1,K,V as [d,t]
        vnat = const.tile([128, NJ, 128], BF16, tag="vnat")  # V natural [j, d]
        ones_sb = const.tile([128, 128], BF16, tag="ones")
        oT = const.tile([128, HL, T], BF16, tag="oT")
        # P-tile ring: [d->kv rows? no: kv rows, slot, head, q cols]
        pbuf = const.tile([128, NSLOT, HL, 512], BF16, tag="pbuf")

        make_identity(nc, ident[:])
        nc.vector.memset(ones_sb[:], 1.0)
        # rotate-half permutation: perm[:, j] = ident[:, (j+64)%128]
        perm_sb = const.tile([128, 128], BF16, tag="perm")
        nc.vector.tensor_copy(perm_sb[:, 0:64], ident[:, 64:128])
        nc.vector.tensor_copy(perm_sb[:, 64:128], ident[:, 0:64])

        kT = qkvT[:, 2, :]

        # ---------- o_proj work queue ----------
        # steps: ("mm", qi, ti, mi) = 2-matmul accumulation into a yp bank
        # + drain into yr[:, mi]; ("store", qi, ti) = DMA yr out.
        opq = deque()

        def enqueue_oproj(qi):
            for tsub in range(4):
                ti = qi * 4 + tsub
                for mi in range(NM):
                    opq.append(("mm", qi, ti, mi))

        yr_live = {}

        def pop_step(tail_idx=None):
            if not opq:
                return
            kind, qi, ti, mi = opq.popleft()
            if tail_idx is None:
                yp = ypsum.tile([128, 512], F32, tag="yp", name=f"yp{ti}_{mi}")
            else:
                ypt = pspool.tile(
                    [128, 2, 512], F32, tag="s", name=f"ypt{ti}_{mi}"
                )
                yp = ypt[:, 0, :]
            for h2 in range(HL):
                nc.tensor.matmul(
                    yp[:],
                    lhsT=oT[:, h2, bass.ts(ti, 128)],
                    rhs=wo_sb[:, h2, bass.ts(mi, 512)],
                    start=(h2 == 0),
                    stop=(h2 == HL - 1),
                )
            if tail_idx is None:
                # mid-kernel: assemble the full row tile, one store per ti —
                # small per-slice stores crowd the sync FIFO ahead of the
                # latency-critical rope-swap DMAs (measured 5-7us PE gaps).
                if mi == 0:
                    yr_live[ti] = yrow.tile(
                        [128, D], BF16, tag="yr", bufs=3, name=f"yr{ti}"
                    )
                yr = yr_live[ti]
                nc.vector.tensor_copy(yr[:, bass.ts(mi, 512)], yp[:])
                if mi == NM - 1:
                    nc.sync.dma_start(y[bass.ts(ti, 128), :], yr_live.pop(ti)[:])
            else:
                # tail: per-slice drains/stores across queues so the last
                # store is 0.125MB, not a 0.5MB row DMA.
                yrs = yrow.tile(
                    [128, 512], BF16, tag="yrs", bufs=4, name=f"yrs{ti}_{mi}"
                )
                if mi % 2 == 1:
                    nc.scalar.copy(yrs[:], yp[:])
                else:
                    nc.vector.tensor_copy(yrs[:], yp[:])
                if (ti * NM + mi) % 2 == 1:
                    nc.scalar.dma_start(
                        y[bass.ts(ti, 128), bass.ts(mi, 512)], yrs[:]
                    )
                else:
                    nc.sync.dma_start(
                        y[bass.ts(ti, 128), bass.ts(mi, 512)], yrs[:]
                    )

        # ---------- softmax denominator ----------
        def emit_lburst(qi, lb, j0, j1):
            # M=1 matmuls in 128x32 column-tiling mode; tiles j=2h+c (head,
            # q-half) run concurrently. Diagonal blocks only touch q >= s0.
            njt = 4 * (qi + 1)
            last_for_c = {0: min(4 * qi + 1, njt - 1), 1: njt - 1}
            for jt in range(j0, j1):
                kd = jt - 4 * qi
                s0 = 128 * kd if kd > 0 else 0
                slot = jt % NSLOT
                for h in range(HL):
                    for c in range(2):
                        c0, c1 = max(s0, 256 * c), 256 * (c + 1)
                        if c0 >= c1:
                            continue
                        j = 2 * h + c
                        nc.tensor.matmul(
                            lb[32 * j : 32 * j + 1, c0:c1],
                            lhsT=ones_sb[:, :1],
                            rhs=pbuf[:, slot, h, c0:c1],
                            start=(jt == 0),
                            stop=(jt == last_for_c[c]),
                            tile_position=(0, 32 * j),
                            skip_group_check=True,
                        )

        norm_state = {}

        def emit_lfinish(qi, lb):
            # HW quirks (probe-verified): DVE custom ops and Pool
            # partition_broadcast only honor base partition 0, so move the
            # four lb quarter-rows to partition 0 via small DMAs first.
            # DVE tensor_copy PSUM->SBUF works at any base partition.
            ltmp = lrowp.tile([128, 512], F32, tag="lr", name=f"ltmp{qi}")
            for h in range(HL):
                for c in range(2):
                    j = 2 * h + c
                    cc = slice(256 * c, 256 * (c + 1))
                    nc.vector.tensor_copy(
                        ltmp[32 * j : 32 * j + 1, cc], lb[32 * j : 32 * j + 1, cc]
                    )
            lbcs = []
            for h in range(HL):
                lrow = lrowp.tile(
                    [128, 512], F32, tag="lrow", bufs=2, name=f"lrow{qi}_{h}"
                )
                for c in range(2):
                    j = 2 * h + c
                    cc = slice(256 * c, 256 * (c + 1))
                    nc.sync.dma_start(
                        lrow[0:1, cc], ltmp[32 * j : 32 * j + 1, cc]
                    )
                rp = lrowp.tile(
                    [128, 512], F32, tag="lrow", bufs=2, name=f"lrp{qi}_{h}"
                )
                nc.vector.reciprocal_approx_fast(rp[0:1, :], lrow[0:1, :])
                lbc = lbcp.tile([128, 512], F32, tag="lbc", name=f"lbc{qi}_{h}")
                nc.gpsimd.partition_broadcast(lbc[:], rp[0:1, :], channels=128)
                lbcs.append(lbc)
            norm_state[qi] = (norm_state.pop(qi), lbcs)

        def emit_lfinish_pe(qi, lb):
            # tail variant: bf16 quarter-row copies + row-tiled rank-1 PE
            # broadcast, reciprocal straight from PSUM (base partition 0 —
            # HW-safe). Shortest serial chain for the end of the kernel.
            lrbf = lrowp.tile([128, 512], BF16, tag="lrbf", bufs=1,
                              name=f"lrbf{qi}")
            for h in range(HL):
                for c in range(2):
                    j = 2 * h + c
                    cc = slice(256 * c, 256 * (c + 1))
                    nc.vector.tensor_copy(
                        lrbf[32 * j : 32 * j + 1, cc], lb[32 * j : 32 * j + 1, cc]
                    )
            lbcs = []
            for h in range(HL):
                bpt = pspool.tile([128, 2, 512], F32, tag="s", name=f"bp{qi}_{h}")
                bp = bpt[:, 0, :]
                for c in range(2):
                    j = 2 * h + c
                    cc = slice(256 * c, 256 * (c + 1))
                    nc.tensor.matmul(
                        bp[:, cc],
                        lhsT=ones_sb[32 * j : 32 * j + 1, 0:128],
                        rhs=lrbf[32 * j : 32 * j + 1, cc],
                        start=True,
                        stop=True,
                        tile_position=(32 * j, 0),
                        skip_group_check=True,
                    )
                lbc = lbcp.tile([128, 512], F32, tag="lbc", name=f"lbcp{qi}_{h}")
                nc.vector.reciprocal_approx_fast(lbc[:], bp[:])
                lbcs.append(lbc)
            norm_state[qi] = (norm_state.pop(qi), lbcs)

        def emit_norm(qi):
            ot, lbcs = norm_state.pop(qi)
            qs = bass.ts(qi, 512)
            for h in range(HL):
                nc.vector.tensor_mul(oT[:, h, qs], ot[h][:], lbcs[h][:])

        # ---------- x prefetch ----------
        xts = {}

        def issue_xt(n):
            xt = xpool.tile([128, KC, 512], BF16, tag="xt", name=f"xt{n}")
            if n == 0:
                # startup, in need-order across the three DMA-issue queues:
                # single-k first pieces (first proj matmul ~6us in), the
                # window-0/1 rope table slices, the remaining k pairs, then
                # wo (first o_proj pop ~50us) and the rope table tails.
                nc.sync.dma_start(xt[0:64, 0:1, :], xp[n, 0:64, 0:1, :])
                nc.scalar.dma_start(xt[64:128, 0:1, :], xp[n, 64:128, 0:1, :])
                nc.gpsimd.dma_start(wqkv_sb[0:64, 0:1], wqkv[0:64, 0:1])
                nc.sync.dma_start(wqkv_sb[64:128, 0:1], wqkv[64:128, 0:1])
                nc.scalar.dma_start(xt[:, 1:2, :], xp[n, :, 1:2, :])
                nc.gpsimd.dma_start(wqkv_sb[:, 1:2], wqkv[:, 1:2])
                nc.sync.dma_start(cos_sb[:, 0:1024], cos2[:, 0:1024])
                nc.scalar.dma_start(sin_sb[:, 0:1024], sinsig[:, 0:1024])
                for kq in range(1, 8):
                    eng = (nc.gpsimd, nc.sync, nc.scalar)[kq % 3]
                    eng.dma_start(
                        xt[:, bass.ts(kq, 2), :], xp[n, :, bass.ts(kq, 2), :]
                    )
                    engw = (nc.sync, nc.scalar, nc.gpsimd)[kq % 3]
                    engw.dma_start(
                        wqkv_sb[:, bass.ts(kq, 2)], wqkv[:, bass.ts(kq, 2)]
                    )
            else:
                for kq in range(4):
                    nc.gpsimd.dma_start(
                        xt[:, bass.ts(kq, 4), :], xp[n, :, bass.ts(kq, 4), :]
                    )
            xts[n] = xt

        for n in range(NQ):
            ns = bass.ts(n, 512)
            if n == 0:
                issue_xt(0)
                nc.sync.dma_start(wo_sb[:, :, 0:1024], wo[:, :, 0:1024])
                nc.scalar.dma_start(wo_sb[:, :, 1024:2048], wo[:, :, 1024:2048])
                nc.sync.dma_start(cos_sb[:, 1024:T], cos2[:, 1024:T])
                nc.scalar.dma_start(sin_sb[:, 1024:T], sinsig[:, 1024:T])
            xt = xts.pop(n)

            # fused q/k/v projection into raw (pre-RoPE) tiles, [d, t]
            qraw = swpool.tile([128, 3, 512], BF16, tag="qraw", name=f"qraw{n}")

            def proj_drain(m, prjm):
                dst = qkvT[:, m, ns] if m == 3 else qraw[:, m, :]
                if m % 2 == 0:
                    nc.vector.tensor_copy(dst, prjm[:])
                else:
                    nc.scalar.copy(dst, prjm[:])

            def rope_swap(i):
                swp = swpool.tile([128, 512], BF16, tag="sw", bufs=3,
                                  name=f"sw{n}_{i}")
                if n >= 2:
                    nc.sync.dma_start(swp[0:64, :], qraw[64:128, i, :])
                    nc.scalar.dma_start(swp[64:128, :], qraw[0:64, i, :])
                else:
                    sw_pt = pspool.tile(
                        [128, 2, 512], F32, tag="s", name=f"swp{n}_{i}"
                    )
                    sw_ps = sw_pt[:, 0, :]
                    nc.tensor.matmul(
                        sw_ps[:],
                        lhsT=perm_sb[:],
                        rhs=qraw[:, i, :],
                        start=True,
                        stop=True,
                    )
                    nc.scalar.copy(swp[:], sw_ps[:])
                return swp

            def rope_mul(i, swp):
                src = qraw[:, i, :]
                dst = qkvT[:, i, ns]
                nc.vector.tensor_mul(dst, src, cos_sb[:, ns])
                nc.vector.tensor_mul(swp[:], swp[:], sin_sb[:, ns])
                nc.vector.tensor_add(dst, dst, swp[:])

            if n == 0:
                # k-outer: each arriving x chunk immediately feeds 4 matmuls
                prjt = [
                    pspool.tile([128, 2, 512], F32, tag="s", name=f"prj{n}_{m}")
                    for m in range(2)
                ]
                prj = [prjt[0][:, 0, :], prjt[0][:, 1, :],
                       prjt[1][:, 0, :], prjt[1][:, 1, :]]
                for k in range(KC):
                    for m in range(4):
                        nc.tensor.matmul(
                            prj[m][:],
                            lhsT=wqkv_sb[:, k, m, :],
                            rhs=xt[:, k, :],
                            start=(k == 0),
                            stop=(k == KC - 1),
                        )
                issue_xt(n + 1)
                for m in range(4):
                    proj_drain(m, prj[m])
                swps = [rope_swap(i) for i in (0, 1, 2)]
                for i in (0, 1, 2):
                    rope_mul(i, swps[i])
            else:
                # m-outer; one o_proj step woven between m-groups
                swps = []
                for m in range(4):
                    prjt = pspool.tile(
                        [128, 2, 512], F32, tag="s", name=f"prj{n}_{m}"
                    )
                    prjm = prjt[:, 0, :]
                    for k in range(KC):
                        nc.tensor.matmul(
                            prjm[:],
                            lhsT=wqkv_sb[:, k, m, :],
                            rhs=xt[:, k, :],
                            start=(k == 0),
                            stop=(k == KC - 1),
                        )
                    if m == 0 and n + 1 < NQ:
                        issue_xt(n + 1)
                    proj_drain(m, prjm)
                    if m < 3:
                        swps.append(rope_swap(m))
                    pop_step()
                for i in (0, 1, 2):
                    rope_mul(i, swps[i])

            # finish l of window n-1 (burst completed at its window end),
            # normalize its O^T, and enqueue its o_proj work.
            if n > 0:
                emit_lfinish(n - 1, lb_prev)
                emit_norm(n - 1)
                enqueue_oproj(n - 1)

            # V^T -> V natural. Early windows need vnat within ~3us of the
            # proj drain (first diagonal O block comes immediately), so use
            # PE transposes there; late windows have >=20us of slack before
            # their diagonal blocks, so XBAR DMA transposes run off PE.
            for jt in range(4 * n, 4 * n + 4):
                if n < 3:
                    tp_t = ypsum.tile([128, 512], F32, tag="yp", name=f"vt{jt}")
                    tp = tp_t.bitcast(BF16)[:, 0:128]
                    nc.tensor.transpose(tp, qkvT[:, 3, bass.ts(jt, 128)], ident[:])
                    nc.vector.tensor_copy(vnat[:, jt, :], tp)
                else:
                    nc.sync.dma_start_transpose(
                        vnat[:, jt, :], qkvT[:, 3, bass.ts(jt, 128)]
                    )

            # ---------- causal attention for q-tile qi=n ----------
            qi = n
            njt = 4 * (qi + 1)
            ot = [
                opsum.tile([128, 512], F32, tag="oacc", name=f"oacc{qi}_{h}")
                for h in range(HL)
            ]
            lb = lpsum.tile([128, 512], F32, tag="lacc", name=f"lacc{qi}")

            def emit_s(jt):
                kd = jt - 4 * qi
                s0 = 128 * kd if kd > 0 else 0
                w = 512 - s0
                slot = jt % NSLOT
                sps = pspool.tile(
                    [128, 2, 512], F32, tag="s", name=f"sps{qi}_{jt}"
                )
                for h in range(HL):
                    nc.tensor.matmul(
                        sps[:, h, :w],
                        lhsT=kT[:, bass.ts(jt, 128)],
                        rhs=qkvT[:, h, qi * 512 + s0 : (qi + 1) * 512],
                        start=True,
                        stop=True,
                    )
                nc.scalar.activation(
                    pbuf[:, slot, :, s0:512], sps[:, :, :w], Exp, scale=SCALE
                )
                if kd >= 0:  # mask the 128-col block straddling the diagonal
                    nc.gpsimd.affine_select(
                        out=pbuf[:, slot, :, s0 : s0 + 128],
                        in_=pbuf[:, slot, :, s0 : s0 + 128],
                        compare_op=mybir.AluOpType.is_ge,
                        fill=0.0,
                        base=0,
                        channel_multiplier=-1,
                        pattern=[[0, 2], [1, 128]],
                    )
                return (jt, s0)

            def emit_ol(jt, s0):
                slot = jt % NSLOT
                for h in range(HL):
                    nc.tensor.matmul(
                        ot[h][:, s0:],
                        lhsT=vnat[:, jt, :],
                        rhs=pbuf[:, slot, h, s0:512],
                        start=(jt == 0),
                        stop=(jt == njt - 1),
                    )

            pend = None
            for jt in range(njt):
                if jt == NSLOT:
                    # first half-window l burst; must precede exp(17),
                    # which reuses ring slot 0.
                    emit_lburst(qi, lb, 0, 16)
                pend_new = emit_s(jt)
                if qi == NQ - 1 and jt == njt - 1:
                    # final window: run blocks 16..30 of the second burst
                    # under the last S stream so only block 31's four
                    # matmuls trail the last O on the tail path.
                    emit_lburst(qi, lb, 16, njt - 1)
                if pend is not None:
                    emit_ol(*pend)
                pend = pend_new
                pop_step()
            emit_ol(*pend)
            if qi == NQ - 1:
                emit_lburst(qi, lb, njt - 1, njt)
            else:
                emit_lburst(qi, lb, 16 if njt > NSLOT else 0, njt)
            norm_state[qi] = ot
            lb_prev = lb

        # ---------- tail: window NQ-1 finish + drain o_proj queue ----------
        emit_lfinish_pe(NQ - 1, lb_prev)
        emit_norm(NQ - 1)
        enqueue_oproj(NQ - 1)
        i = 0
        while opq:
            pop_step(tail_idx=i)
            i += 1


_CACHE = {}


def _get_program():
    if "nc" not in _CACHE:
        nc = bacc.Bacc(
            "TRN2", target_bir_lowering=False, debug=False, num_devices=N_CORES
        )
        _build(nc)
        nc.compile()
        _CACHE["nc"] = nc
    return _CACHE["nc"]


def _rope_tables():
    inv_freq = 1.0 / (ROPE_BASE ** (np.arange(64, dtype=np.float64) / 64))
    ang = np.arange(T, dtype=np.float64)[:, None] * inv_freq[None, :]  # [T, 64]
    cos = np.cos(ang).T  # [64, T]
    sin = np.sin(ang).T
    cos2 = np.concatenate([cos, cos], axis=0).astype(NPBF16)
    sinsig = np.concatenate([-sin, sin], axis=0).astype(NPBF16)
    return cos2, sinsig


def kernel(x, Wq, Wk, Wv, Wo):
    x = np.asarray(x, dtype=np.float32)
    Wq = np.asarray(Wq, dtype=np.float32)
    Wk = np.asarray(Wk, dtype=np.float32)
    Wv = np.asarray(Wv, dtype=np.float32)
    Wo = np.asarray(Wo, dtype=np.float32)

    # x[t, c] -> xp[n, p, k, j] = x[n*512+j, k*128+p]; contiguous per partition.
    xp = np.ascontiguousarray(
        x.reshape(T, D).reshape(NQ, 512, KC, 128).transpose(0, 3, 2, 1)
    ).astype(NPBF16)
    cos2, sinsig = _rope_tables()

    in_maps = []
    for c in range(N_CORES):
        h0, h1 = 2 * c, 2 * c + 1
        kv = c // 2
        wqkv_c = np.concatenate(
            [
                Wq[:, h0 * DH:(h0 + 1) * DH],
                Wq[:, h1 * DH:(h1 + 1) * DH],
                Wk[:, kv * DH:(kv + 1) * DH],
                Wv[:, kv * DH:(kv + 1) * DH],
            ],
            axis=1,
        )  # [D, 512]
        wqkv_pre = np.ascontiguousarray(
            wqkv_c.reshape(KC, 128, 4, 128).transpose(1, 0, 2, 3)
        ).astype(NPBF16)
        wo_pre = np.ascontiguousarray(
            np.stack(
                [Wo[h0 * DH:(h0 + 1) * DH, :], Wo[h1 * DH:(h1 + 1) * DH, :]], axis=0
            ).transpose(1, 0, 2)
        ).astype(NPBF16)
        in_maps.append(
            {
                "xp": xp,
                "wqkv": wqkv_pre,
                "wo": wo_pre,
                "cos2": cos2,
                "sinsig": sinsig,
            }
        )

    nc = _get_program()
    res = run_bass_kernel_spmd(nc, in_maps, list(range(N_CORES)))
    out = np.zeros((T, D), dtype=np.float32)
    for c in range(N_CORES):
        out += res.results[c]["y"].astype(np.float32)
    return out.reshape(B, T, D)
